# revision 22
# baseline (speedup 1.0000x reference)
"""Trainium2 Bass kernel for margin-ranking + weighted-BCE loss pair.

Math
----
Labels are binary {0,1}, so dl = l_i - l_j is 0 for same-label pairs and
+-1 for cross-label pairs:

  same-label pair:  prod = 0          -> contributes relu(m)
  cross-label pair: prod = p_pos - p_neg
                                      -> contributes relu(m - p_pos + p_neg)

  margin_loss = [ N_eq * relu(m) + sum_{a in pos, b in neg} relu(c_a + p_b) ] / B
  with c_a = m - p_a,  N_eq = C(n0,2) + C(n1,2).

The cross-label sum is an OUTER SUM: each [128, F] tile is
relu(neg_row_broadcast + c_a[P,1]) -- ~20M relu evals vs ~67M in the
all-pairs rank-4 matmul form.

Measured HW rates drive the engine split. DVE's plain tensor_scalar on
bf16 SBUF runs at 4 elem/cycle/lane, but the fused accumulate variant
(TENSOR_SCALAR_CACHE_REDUCE) and every other reducer run at ~1x. So DVE
*generates* relu tiles at 4x and otherwise-idle engines reduce them:

  k=0..3 : DVE relu 4x -> PE ones-matmul strips into PSUM (accumulated
           across chunks; raw [1, FCOLS] strip sums DMA'd to host)
  k=4,5  : ScalarE fused activation(Relu, bias=c, accum_out)
  k=6,7  : DVE relu 4x -> GpSimd tensor_reduce
  k=8    : DVE fused tensor_scalar(max, add, accum_out): computes
           sum_b max(p_b, -c_a) = sum relu - F*c_a; the F*csum(c)
           correction is applied on the host from its own staging array.

Distribution: positives padded to NP=4608 (4 row groups x 9 chunks x
128), negatives to NN=4352 (2 col groups x 2176). Pads (+16/-16) make
every pad row contribute exactly 0 after correction. BCE on a
1024-element shard per core: (1-t)z + (1+(pw-1)t)*ln(1+exp(-z)) with
dummy exp/ln issued early so ACT table loads hide under the input DMA.
Host sums per-core partials and adds the N_eq term.
"""

import numpy as np
import ml_dtypes

import concourse.bacc as bacc
import concourse.bass as bass
import concourse.mybir as mybir
import concourse.tile as tile
from concourse.bass_utils import run_bass_kernel_spmd

B = 8192
NCORES = 8
NP = 4608                  # padded positive count (4 row groups x 1152)
NN = 4352                  # padded negative count (2 col groups x 2176)
NROWG = 4
NCOLG = 2
RROWS = NP // NROWG        # 1152 positive rows per core
T = RROWS // 128           # 9 chunks per core
FCOLS = NN // NCOLG        # 2176 negative cols per core
PAD_POS = 16.0
PAD_NEG = -16.0
P = 128
BCE_N = B // NCORES        # 1024 -> [128, 8]
BCE_F = BCE_N // P         # 8
NWARM = 6                  # PE clock warm matmuls during the DMA window

# chunk -> strategy split (tune against trace):
N_PE = 4                   # k < N_PE: DVE relu -> PE strip reduce
N_ACT = 2                  # next: ScalarE fused
N_POOL = 2                 # next: DVE relu -> GpSimd reduce
N_DVE = T - N_PE - N_ACT - N_POOL  # rest: DVE fused (csum-corrected)
STRIPS = [(0, 512), (512, 1024), (1024, 1536), (1536, 2048), (2048, 2176)]

# out cols: 0:N_DVE dve-fused | acc_a | acc_p | bce1 | bce2
OUTC = N_DVE + N_ACT + N_POOL + 2
PCK = T + 2 * BCE_F + 1    # packed f32 input: posm | z | t | pw

f32 = mybir.dt.float32
bf16 = mybir.dt.bfloat16


def _build_program(margin: float):
    from contextlib import ExitStack

    nc = bacc.Bacc("TRN2", target_bir_lowering=False, debug=False,
                   num_devices=NCORES)
    Relu = mybir.ActivationFunctionType.Relu
    Exp = mybir.ActivationFunctionType.Exp
    Ln = mybir.ActivationFunctionType.Ln
    add = mybir.AluOpType.add
    mult = mybir.AluOpType.mult
    amax = mybir.AluOpType.max

    neg_d = nc.dram_tensor("negr", [P, FCOLS], bf16, kind="ExternalInput")
    pck_d = nc.dram_tensor("pck", [P, PCK], f32, kind="ExternalInput")
    out_d = nc.dram_tensor("out", [1, OUTC], f32, kind="ExternalOutput")
    strip_d = nc.dram_tensor("strip", [3, 1024], f32, kind="ExternalOutput")

    with tile.TileContext(nc) as tc, ExitStack() as ctx:
        small = ctx.enter_context(tc.tile_pool(name="small", bufs=1))
        psum = ctx.enter_context(
            tc.tile_pool(name="psum", bufs=1, space=bass.MemorySpace.PSUM))

        # ---- dependency-free setup (runs during DMA) ---------------------
        negrep = small.tile([P, FCOLS], bf16, tag="negrep")
        pck = small.tile([P, PCK], f32, tag="pck")
        nc.sync.dma_start(out=negrep[0:64, :], in_=neg_d[0:64, :])
        nc.scalar.dma_start(out=negrep[64:P, :], in_=neg_d[64:P, :])
        nc.gpsimd.dma_start(out=pck[:, :], in_=pck_d[:, :])

        posm = pck[:, 0:T]
        zt = pck[:, T:T + BCE_F]
        tt = pck[:, T + BCE_F:T + 2 * BCE_F]
        pwt = pck[:, T + 2 * BCE_F:T + 2 * BCE_F + 1]

        ones1 = small.tile([P, 1], f32, tag="ones1")
        ones32b = small.tile([P, 32], bf16, tag="ones32b")
        wtile = small.tile([P, 512], bf16, tag="wtile")
        tiny = small.tile([1, 2], f32, tag="tiny")
        nc.gpsimd.memset(ones1[:, :], 1.0)
        nc.gpsimd.memset(ones32b[:, :], 1.0)
        nc.vector.memset(wtile[:, :], 1.0)
        nc.vector.memset(tiny[:, :], 1.0)

        # ACT table pre-load: dummy exp/ln before any real dependency
        nc.scalar.activation(tiny[:, 0:1], tiny[:, 0:1], Exp, scale=-1.0)
        nc.scalar.activation(tiny[:, 1:2], tiny[:, 1:2], Ln, bias=1.0)

        # PE clock pre-warm
        pwarm = psum.tile([P, 512], f32, tag="pwarm")
        for _ in range(NWARM):
            nc.tensor.matmul(pwarm[:, :], wtile[:, 0:P], wtile[:, :],
                             start=True, stop=True)

        # pstrip: strips 0-2 at partition 0/32/64 cols 0:512; strip 3 at
        # partition 0 cols 512:1024; strip 4 at partition 32 cols 512:640.
        # Two dummy writes cover the remaining cells the strided copy
        # reads (host ignores them).
        pstrip = psum.tile([P, 1024], f32, tag="pstrip")
        nc.tensor.matmul(pstrip[32:64, 640:1024], ones32b[:, :],
                         wtile[:, 0:384], start=True, stop=True)
        nc.tensor.matmul(pstrip[64:96, 512:1024], ones32b[:, :],
                         wtile[:, 0:512], start=True, stop=True)

        # c_a = m - p_a and mc_a = -c_a, [128, T] f32 (exact)
        cmat = small.tile([P, T], f32, tag="cmat")
        mcmat = small.tile([P, T], f32, tag="mcmat")
        nc.vector.tensor_scalar(cmat[:, :], posm, -1.0, float(margin),
                                mult, add)
        nc.vector.tensor_scalar_add(mcmat[:, :], posm, -float(margin))

        # ---- BCE: (1-t)z + (1+(pw-1)t) * ln(1+exp(-z)) -------------------
        ev = small.tile([P, BCE_F], f32, tag="ev")
        sp = small.tile([P, BCE_F], f32, tag="sp")
        wv = small.tile([P, BCE_F], f32, tag="wv")
        tz = small.tile([P, BCE_F], f32, tag="tz")
        pwm1 = small.tile([P, 1], f32, tag="pwm1")
        dmy1 = small.tile([P, BCE_F], f32, tag="dmy1")
        dmy2 = small.tile([P, BCE_F], f32, tag="dmy2")
        acc1 = small.tile([P, 1], f32, tag="acc1")
        acc2 = small.tile([P, 1], f32, tag="acc2")

        nc.gpsimd.tensor_scalar_add(pwm1[:, :], pwt, -1.0)
        nc.gpsimd.tensor_scalar(wv[:, :], tt, pwm1[:, 0:1], 1.0, mult, add)
        nc.gpsimd.tensor_mul(tz[:, :], tt, zt)
        nc.scalar.activation(ev[:, :], zt, Exp, scale=-1.0)
        nc.scalar.activation(sp[:, :], ev[:, :], Ln, bias=1.0)
        nc.gpsimd.tensor_mul(dmy1[:, :], wv[:, :], sp[:, :])
        nc.gpsimd.tensor_sub(dmy2[:, :], zt, tz[:, :])
        nc.vector.tensor_reduce(acc1[:, :], dmy1[:, :],
                                axis=mybir.AxisListType.X, op=add)
        nc.vector.tensor_reduce(acc2[:, :], dmy2[:, :],
                                axis=mybir.AxisListType.X, op=add)

        # ---- the 9 chunks ------------------------------------------------
        scr_pe = [small.tile([P, FCOLS], bf16, name=f"scr_pe{i}",
                              tag=f"scr_pe{i}") for i in range(2)]
        scr_po = [small.tile([P, FCOLS], bf16, name=f"scr_po{i}",
                              tag=f"scr_po{i}") for i in range(2)]
        scr_a = small.tile([P, FCOLS], bf16, tag="scr_a")
        scr_d = small.tile([P, FCOLS], bf16, tag="scr_d")
        acc_a = small.tile([P, N_ACT], f32, tag="acc_a")
        acc_d = small.tile([P, max(N_DVE, 1)], f32, tag="acc_d")
        outt = small.tile([1, OUTC], f32, tag="outt")

        def relu_dve(k, scr):
            nc.vector.tensor_scalar(scr[:, :], negrep[:, :],
                                    cmat[:, k:k + 1], 0.0, add, amax)

        STRIP_OUT = [((0, 32), (0, 512)), ((32, 64), (0, 512)),
                     ((64, 96), (0, 512)), ((0, 32), (512, 1024)),
                     ((32, 64), (512, 640))]

        def pe_reduce(i, scr):
            st = (i == 0)
            sp = (i == N_PE - 1)
            for s, ((p0, p1), (c0, c1)) in enumerate(STRIP_OUT):
                nc.tensor.matmul(pstrip[p0:p1, c0:c1], ones32b[:, :],
                                 scr[:, 512 * s:512 * s + (c1 - c0)],
                                 start=st, stop=sp, skip_group_check=True)

        # DVE stream order interleaves consumers so no engine starves.
        pe_ks = list(range(N_PE))
        po_ks = list(range(N_PE + N_ACT, N_PE + N_ACT + N_POOL))
        dve_ks = list(range(N_PE + N_ACT + N_POOL, T))
        act_ks = list(range(N_PE, N_PE + N_ACT))

        order = []
        # producer order: pe0, pe1, po0, pe2, po1, pe3, then fused
        mix = [("pe", 0), ("pe", 1), ("po", 0), ("pe", 2), ("po", 1),
               ("pe", 3)]
        for kind, i in mix:
            if kind == "pe" and i < len(pe_ks):
                order.append(("pe", i, pe_ks[i]))
            elif kind == "po" and i < len(po_ks):
                order.append(("po", i, po_ks[i]))
        for i, k in enumerate(dve_ks):
            order.append(("dve", i, k))

        for kind, i, k in order:
            if kind == "pe":
                scr = scr_pe[i % 2]
                relu_dve(k, scr)
                pe_reduce(i, scr)
            elif kind == "po":
                scr = scr_po[i % 2]
                relu_dve(k, scr)
                c = N_DVE + N_ACT + i
                nc.gpsimd.tensor_reduce(outt[0:1, c:c + 1], scr[:, :],
                                        axis=mybir.AxisListType.XYZWC,
                                        op=add)
            else:
                nc.vector.tensor_scalar(scr_d[:, :], negrep[:, :],
                                        mcmat[:, k:k + 1], 0.0, amax, add,
                                        accum_out=acc_d[:, i:i + 1])
        for i, k in enumerate(act_ks):
            nc.scalar.activation(scr_a[:, :], negrep[:, :], Relu,
                                 bias=cmat[:, k:k + 1],
                                 accum_out=acc_a[:, i:i + 1])

        # ---- partition reduction + output --------------------------------
        NDA = N_DVE + N_ACT
        pfin = psum.tile([1, NDA + 2], f32, tag="pfin")
        c0 = 0
        for acc, w in ((acc_d, N_DVE), (acc_a, N_ACT), (acc1, 1), (acc2, 1)):
            if w:
                nc.tensor.matmul(pfin[:, c0:c0 + w], ones1[:, :],
                                 acc[:, 0:w], start=True, stop=True)
                c0 += w
        nc.scalar.copy(outt[:, 0:NDA], pfin[:, 0:NDA])
        nc.scalar.copy(outt[:, OUTC - 2:OUTC], pfin[:, NDA:NDA + 2])
        strip_sb = small.tile([96, 1024], f32, tag="strip_sb")
        nc.scalar.copy(strip_sb[:, :], pstrip[0:96, 0:1024])
        nc.sync.dma_start(out=out_d[:, :], in_=outt[:, :])
        nc.sync.dma_start(out=strip_d[:, :], in_=strip_sb[0:96:32, :])

    nc.compile()
    return nc


_programs: dict = {}


def _get_program(margin: float):
    key = margin
    if key not in _programs:
        _programs[key] = _build_program(margin)
    return _programs[key]


def _make_in_maps(preds, labels, logits, targets, pos_weight):
    p = np.ascontiguousarray(np.asarray(preds, np.float32))
    l = np.ascontiguousarray(np.asarray(labels, np.float32))
    z = np.ascontiguousarray(np.asarray(logits, np.float32))
    tg = np.ascontiguousarray(np.asarray(targets, np.float32))
    pw = float(np.asarray(pos_weight, np.float32).reshape(-1)[0])
    ndt = ml_dtypes.bfloat16

    mask = l >= 0.5
    pos = p[mask]
    neg = p[~mask]
    n1, n0 = len(pos), len(neg)
    assert n1 <= NP and n0 <= NN, (n1, n0)
    posf = np.full(NP, PAD_POS, np.float32)
    posf[:n1] = pos
    negf = np.full(NN, PAD_NEG, np.float32)
    negf[:n0] = neg
    negb = negf.astype(ndt)

    in_maps = []
    for c in range(NCORES):
        r, j = divmod(c, NCOLG)
        posm = np.ascontiguousarray(
            posf[r * RROWS:(r + 1) * RROWS].reshape(T, P).T)
        negrep = np.ascontiguousarray(
            np.broadcast_to(negb[j * FCOLS:(j + 1) * FCOLS], (P, FCOLS)))
        pck = np.empty((P, PCK), np.float32)
        pck[:, 0:T] = posm
        pck[:, T:T + BCE_F] = z[BCE_N * c: BCE_N * (c + 1)].reshape(P, BCE_F)
        pck[:, T + BCE_F:T + 2 * BCE_F] = \
            tg[BCE_N * c: BCE_N * (c + 1)].reshape(P, BCE_F)
        pck[:, T + 2 * BCE_F] = pw
        in_maps.append({"negr": negrep, "pck": pck})
    return in_maps, n0, n1, posf


def _combine(outs, strips, margin: float, n0: int, n1: int,
             posf: np.ndarray) -> np.ndarray:
    # outs: [NCORES, 1, OUTC]; strips: [NCORES, 3, 1024]: strips 0-2 in
    # rows 0-2 cols 0:512, strip 3 in row 0 cols 512:1024, strip 4 in
    # row 1 cols 512:640; everything else is dummy fill.
    st = strips.astype(np.float64)
    s_cross = (float(st[:, :, 0:512].sum()) +
               float(st[:, 0, 512:1024].sum()) +
               float(st[:, 1, 512:640].sum()))
    o = outs[:, 0, :].astype(np.float64)
    s_cross += float(o[:, N_DVE:N_DVE + N_ACT + N_POOL].sum())
    # DVE-fused chunks: add back F * csum(c) per (core, chunk)
    for c in range(NCORES):
        r = c // NCOLG
        for i in range(N_DVE):
            k = N_PE + N_ACT + N_POOL + i
            chunk = posf[r * RROWS + k * P: r * RROWS + (k + 1) * P]
            csum = float(margin) * P - float(chunk.astype(np.float64).sum())
            s_cross += float(o[c, i]) + FCOLS * csum
    s_bce = float(o[:, OUTC - 2:].sum())
    n_eq = 0.5 * (n0 * (n0 - 1) + n1 * (n1 - 1))
    margin_loss = (s_cross + n_eq * max(float(margin), 0.0)) / B
    bce_loss = s_bce / B
    return np.array([margin_loss, bce_loss], dtype=np.float32)


def _run(inputs: dict, trace: bool = False, **spmd_kwargs):
    m = float(np.asarray(inputs["margin"]))
    nc = _get_program(m)
    in_maps, n0, n1, posf = _make_in_maps(inputs["preds"], inputs["labels"],
                                          inputs["logits"], inputs["targets"],
                                          inputs["pos_weight"])
    res = run_bass_kernel_spmd(nc, in_maps, core_ids=list(range(NCORES)),
                               trace=trace, **spmd_kwargs)
    outs = np.stack([np.asarray(r["out"], np.float32) for r in res.results])
    strips = np.stack([np.asarray(r["strip"], np.float32)
                       for r in res.results])
    return _combine(outs, strips, m, n0, n1, posf), res


def kernel(preds, labels, logits, targets, pos_weight, margin):
    out, _ = _run(dict(preds=preds, labels=labels, logits=logits,
                       targets=targets, pos_weight=pos_weight,
                       margin=margin))
    return out


# revision 23
# speedup vs baseline: 1.4858x; 1.4858x over previous
"""Trainium2 Bass kernel for margin-ranking + weighted-BCE loss pair.

Math
----
Labels are binary {0,1}, so dl = l_i - l_j is 0 for same-label pairs and
+-1 for cross-label pairs:

  same-label pair:  prod = 0          -> contributes relu(m)
  cross-label pair: prod = p_pos - p_neg
                                      -> contributes relu(m - p_pos + p_neg)

  margin_loss = [ N_eq * relu(m) + sum_{a in pos, b in neg} relu(c_a + p_b) ] / B
  with c_a = m - p_a,  N_eq = C(n0,2) + C(n1,2).

The cross-label sum is an OUTER SUM: each [128, F] tile is
relu(neg_row_broadcast + c_a[P,1]) with free-dim accumulation -- ~19M
relu evals vs ~67M in the all-pairs rank-4 matmul form, and no matmul /
PSUM / PE-clock machinery at all.

Measured on HW: any elementwise op with a per-partition scalar AP runs
at 1 elem/cycle/lane on both DVE and ACT (the 4x/2x perf modes only
engage for immediate scalars), PE strip-reduces run cold, and GpSimd
reduces are ~10us. So the optimal simple structure is just: split the
17 fused relu+accum chunks across DVE (tensor_scalar max/add with
accum, corrected by a host-side csum term) and ScalarE (activation
Relu with per-partition bias and accum_out), with BCE's elementwise on
the otherwise-idle GpSimd and exp/ln tables pre-loaded via dummy
activations during the input DMA.

Distribution: positives padded to NP=4352 (2 row groups x 17 x 128),
negatives to NN=4352 (4 col groups x 1088). Pads (+16/-16) contribute
exactly 0 after the csum correction. Host sums per-core partials and
adds the N_eq term.
"""

import numpy as np
import ml_dtypes

import concourse.bacc as bacc
import concourse.bass as bass
import concourse.mybir as mybir
import concourse.tile as tile
from concourse.bass_utils import run_bass_kernel_spmd

B = 8192
NCORES = 8
NP = 4352                  # padded positive count (2 row groups x 2176)
NN = 4352                  # padded negative count (4 col groups x 1088)
NROWG = 2
NCOLG = 4
RROWS = NP // NROWG        # 2176 positive rows per core
T = RROWS // 128           # 17 chunks per core
FCOLS = NN // NCOLG        # 1088 negative cols per core
PAD_POS = 16.0
PAD_NEG = -16.0
P = 128
BCE_N = B // NCORES        # 1024 -> [128, 8]
BCE_F = BCE_N // P         # 8

# chunk split: first ND chunks DVE-fused (max/add + csum correction),
# rest ACT-fused (activation Relu + bias).
ND = 9
NA = T - ND                # 8
OUTC = T + 2               # T margin partials + 2 bce partials
PCK = T + 2 * BCE_F + 1    # packed f32 input: posm | z | t | pw

f32 = mybir.dt.float32
bf16 = mybir.dt.bfloat16


def _build_program(margin: float):
    from contextlib import ExitStack

    nc = bacc.Bacc("TRN2", target_bir_lowering=False, debug=False,
                   num_devices=NCORES)
    Relu = mybir.ActivationFunctionType.Relu
    Exp = mybir.ActivationFunctionType.Exp
    Ln = mybir.ActivationFunctionType.Ln
    add = mybir.AluOpType.add
    mult = mybir.AluOpType.mult
    amax = mybir.AluOpType.max

    neg_d = nc.dram_tensor("negr", [P, FCOLS], bf16, kind="ExternalInput")
    pck_d = nc.dram_tensor("pck", [P, PCK], f32, kind="ExternalInput")
    out_d = nc.dram_tensor("out", [1, OUTC], f32, kind="ExternalOutput")

    with tile.TileContext(nc) as tc, ExitStack() as ctx:
        small = ctx.enter_context(tc.tile_pool(name="small", bufs=1))
        psum = ctx.enter_context(
            tc.tile_pool(name="psum", bufs=1, space=bass.MemorySpace.PSUM))

        # ---- input DMAs across three queues ------------------------------
        negrep = small.tile([P, FCOLS], bf16, tag="negrep")
        pck = small.tile([P, PCK], f32, tag="pck")
        nc.sync.dma_start(out=negrep[0:64, :], in_=neg_d[0:64, :])
        nc.scalar.dma_start(out=negrep[64:P, :], in_=neg_d[64:P, :])
        nc.gpsimd.dma_start(out=pck[:, :], in_=pck_d[:, :])

        posm = pck[:, 0:T]
        zt = pck[:, T:T + BCE_F]
        tt = pck[:, T + BCE_F:T + 2 * BCE_F]
        pwt = pck[:, T + 2 * BCE_F:T + 2 * BCE_F + 1]

        ones1 = small.tile([P, 1], f32, tag="ones1")
        tiny = small.tile([1, 2], f32, tag="tiny")
        nc.gpsimd.memset(ones1[:, :], 1.0)
        nc.vector.memset(tiny[:, :], 1.0)

        # ACT table pre-load during the DMA window
        nc.scalar.activation(tiny[:, 0:1], tiny[:, 0:1], Exp, scale=-1.0)
        nc.scalar.activation(tiny[:, 1:2], tiny[:, 1:2], Ln, bias=1.0)

        # c_a = m - p_a (ACT bias) and mc_a = p_a - m (DVE max operand)
        cmat = small.tile([P, T], f32, tag="cmat")
        mcmat = small.tile([P, T], f32, tag="mcmat")
        nc.vector.tensor_scalar(cmat[:, :], posm, -1.0, float(margin),
                                mult, add)
        nc.vector.tensor_scalar_add(mcmat[:, :], posm, -float(margin))

        # ---- BCE: (1-t)z + (1+(pw-1)t) * ln(1+exp(-z)) -------------------
        ev = small.tile([P, BCE_F], f32, tag="ev")
        sp = small.tile([P, BCE_F], f32, tag="sp")
        wv = small.tile([P, BCE_F], f32, tag="wv")
        tz = small.tile([P, BCE_F], f32, tag="tz")
        pwm1 = small.tile([P, 1], f32, tag="pwm1")
        dmy1 = small.tile([P, BCE_F], f32, tag="dmy1")
        dmy2 = small.tile([P, BCE_F], f32, tag="dmy2")
        acc1 = small.tile([P, 1], f32, tag="acc1")
        acc2 = small.tile([P, 1], f32, tag="acc2")

        nc.gpsimd.tensor_scalar_add(pwm1[:, :], pwt, -1.0)
        nc.gpsimd.tensor_scalar(wv[:, :], tt, pwm1[:, 0:1], 1.0, mult, add)
        nc.gpsimd.tensor_mul(tz[:, :], tt, zt)
        nc.scalar.activation(ev[:, :], zt, Exp, scale=-1.0)
        nc.scalar.activation(sp[:, :], ev[:, :], Ln, bias=1.0)
        nc.gpsimd.tensor_mul(dmy1[:, :], wv[:, :], sp[:, :])
        nc.gpsimd.tensor_sub(dmy2[:, :], zt, tz[:, :])
        nc.vector.tensor_reduce(acc1[:, :], dmy1[:, :],
                                axis=mybir.AxisListType.X, op=add)
        nc.vector.tensor_reduce(acc2[:, :], dmy2[:, :],
                                axis=mybir.AxisListType.X, op=add)

        # ---- the 17 fused relu+accumulate chunks -------------------------
        scr_d = small.tile([P, FCOLS], bf16, tag="scr_d")
        scr_a = small.tile([P, FCOLS], bf16, tag="scr_a")
        acc_d = small.tile([P, ND], f32, tag="acc_d")
        acc_a = small.tile([P, NA], f32, tag="acc_a")

        for k in range(T):
            if k < ND:
                nc.vector.tensor_scalar(scr_d[:, :], negrep[:, :],
                                        mcmat[:, k:k + 1], 0.0, amax, add,
                                        accum_out=acc_d[:, k:k + 1])
            else:
                nc.scalar.activation(scr_a[:, :], negrep[:, :], Relu,
                                     bias=cmat[:, k:k + 1],
                                     accum_out=acc_a[:, k - ND:k - ND + 1])

        # ---- partition reduction via ones-matmuls + output ---------------
        pfin = psum.tile([1, OUTC], f32, tag="pfin")
        c0 = 0
        for acc, w in ((acc_d, ND), (acc_a, NA), (acc1, 1), (acc2, 1)):
            nc.tensor.matmul(pfin[:, c0:c0 + w], ones1[:, :], acc[:, 0:w],
                             start=True, stop=True)
            c0 += w
        outt = small.tile([1, OUTC], f32, tag="outt")
        nc.scalar.copy(outt[:, :], pfin[:, :])
        nc.sync.dma_start(out=out_d[:, :], in_=outt[:, :])

    nc.compile()
    return nc


_programs: dict = {}


def _get_program(margin: float):
    key = margin
    if key not in _programs:
        _programs[key] = _build_program(margin)
    return _programs[key]


def _make_in_maps(preds, labels, logits, targets, pos_weight):
    p = np.ascontiguousarray(np.asarray(preds, np.float32))
    l = np.ascontiguousarray(np.asarray(labels, np.float32))
    z = np.ascontiguousarray(np.asarray(logits, np.float32))
    tg = np.ascontiguousarray(np.asarray(targets, np.float32))
    pw = float(np.asarray(pos_weight, np.float32).reshape(-1)[0])
    ndt = ml_dtypes.bfloat16

    mask = l >= 0.5
    pos = p[mask]
    neg = p[~mask]
    n1, n0 = len(pos), len(neg)
    assert n1 <= NP and n0 <= NN, (n1, n0)
    posf = np.full(NP, PAD_POS, np.float32)
    posf[:n1] = pos
    negf = np.full(NN, PAD_NEG, np.float32)
    negf[:n0] = neg
    negb = negf.astype(ndt)

    in_maps = []
    for c in range(NCORES):
        r, j = divmod(c, NCOLG)
        posm = np.ascontiguousarray(
            posf[r * RROWS:(r + 1) * RROWS].reshape(T, P).T)
        negrep = np.ascontiguousarray(
            np.broadcast_to(negb[j * FCOLS:(j + 1) * FCOLS], (P, FCOLS)))
        pck = np.empty((P, PCK), np.float32)
        pck[:, 0:T] = posm
        pck[:, T:T + BCE_F] = z[BCE_N * c: BCE_N * (c + 1)].reshape(P, BCE_F)
        pck[:, T + BCE_F:T + 2 * BCE_F] = \
            tg[BCE_N * c: BCE_N * (c + 1)].reshape(P, BCE_F)
        pck[:, T + 2 * BCE_F] = pw
        in_maps.append({"negr": negrep, "pck": pck})
    return in_maps, n0, n1, posf


def _combine(outs, margin: float, n0: int, n1: int,
             posf: np.ndarray) -> np.ndarray:
    # outs: [NCORES, 1, OUTC]: T margin partials then 2 bce partials.
    # DVE chunks (k < ND) computed sum max(p_b, -c_a): add back F*csum(c).
    o = outs[:, 0, :].astype(np.float64)
    s_cross = float(o[:, :T].sum())
    for c in range(NCORES):
        r = c // NCOLG
        for k in range(ND):
            chunk = posf[r * RROWS + k * P: r * RROWS + (k + 1) * P]
            csum = float(margin) * P - float(chunk.astype(np.float64).sum())
            s_cross += FCOLS * csum
    s_bce = float(o[:, T:].sum())
    n_eq = 0.5 * (n0 * (n0 - 1) + n1 * (n1 - 1))
    margin_loss = (s_cross + n_eq * max(float(margin), 0.0)) / B
    bce_loss = s_bce / B
    return np.array([margin_loss, bce_loss], dtype=np.float32)


def _run(inputs: dict, trace: bool = False, **spmd_kwargs):
    m = float(np.asarray(inputs["margin"]))
    nc = _get_program(m)
    in_maps, n0, n1, posf = _make_in_maps(inputs["preds"], inputs["labels"],
                                          inputs["logits"], inputs["targets"],
                                          inputs["pos_weight"])
    res = run_bass_kernel_spmd(nc, in_maps, core_ids=list(range(NCORES)),
                               trace=trace, **spmd_kwargs)
    outs = np.stack([np.asarray(r["out"], np.float32) for r in res.results])
    return _combine(outs, m, n0, n1, posf), res


def kernel(preds, labels, logits, targets, pos_weight, margin):
    out, _ = _run(dict(preds=preds, labels=labels, logits=logits,
                       targets=targets, pos_weight=pos_weight,
                       margin=margin))
    return out
